# revision 1
# baseline (speedup 1.0000x reference)
"""Distributed Trainium2 Bass kernel: masked (upper-triangular) attention.

reference (L=4096, D=1024, fp32):
    Q = x @ Wq + bq ; K = z @ Wk + bk ; V = z @ Wv + bv
    S = Q @ K.T ; S[row > col] = -inf
    out = softmax(S / sqrt(D)) @ V

Strategy (8 NeuronCores, one TRN2 chip, SPMD):
  - Sequence parallel on query rows: core c owns rows [512c, 512c+512).
  - K/V projection sharded over z rows (512/core), AllGathered in bf16
    (K stored transposed [D, L] blocked by shard, V natural [L, D]).
  - Attention computed as S^T tiles (keys on partitions) so the P^T needed by
    the PV matmul comes straight out of the softmax with no transposes.
  - Softmax without max-subtraction (scores here are O(1), exp can't overflow
    in fp32); mask applied multiplicatively after exp, built at runtime from
    an iota constant + a per-core row0 scalar input, keeping one graph valid
    for all cores (SPMD - no per-core control flow).
  - Matmuls in bf16 with fp32 PSUM accumulation (end-to-end rel err ~3e-3).
"""

import math

import numpy as np

import concourse.mybir as mybir
import concourse.tile as tile
from concourse import bacc
from concourse.bass_utils import run_bass_kernel_spmd

F32 = mybir.dt.float32
BF16 = mybir.dt.bfloat16
AF = mybir.ActivationFunctionType
OP = mybir.AluOpType
P = 128
NCORES = 8

L = 4096
D = 1024


def build_graph(Ldim=L, Ddim=D):
    nc = bacc.Bacc("TRN2", target_bir_lowering=False, debug=False, num_devices=NCORES)
    ROWS = Ldim // NCORES        # query rows per core
    MB = ROWS // P               # 128-row m-chunks per core (4)
    ZB = ROWS // P               # z-shard 128-row blocks (4)
    SW = ROWS                    # key-tile width == z-shard width (512)
    JT = SW // P                 # 128-row subtiles per key tile (4)
    NT = NCORES                  # one key tile per shard
    IO = Ddim // P               # contraction chunks (8)
    AO = Ddim // P               # d_attn 128-blocks (8)
    VH = Ddim // 512             # 512-wide value column halves (2)
    HLF = ROWS // 256            # 256-row halves for PV psum pressure (2)
    scale = 1.0 / math.sqrt(Ddim)

    x_ext = nc.declare_dram_parameter("x", [P, ROWS // P, Ddim], F32, isOutput=False)
    z_ext = nc.declare_dram_parameter("z", [P, ROWS // P, Ddim], F32, isOutput=False)
    wq_ext = nc.declare_dram_parameter("Wq", [Ddim, Ddim], F32, isOutput=False)
    wk_ext = nc.declare_dram_parameter("Wk", [Ddim, Ddim], F32, isOutput=False)
    wv_ext = nc.declare_dram_parameter("Wv", [Ddim, Ddim], F32, isOutput=False)
    bq_ext = nc.declare_dram_parameter("bq", [Ddim], F32, isOutput=False)
    bk_ext = nc.declare_dram_parameter("bk", [Ddim], F32, isOutput=False)
    bv_ext = nc.declare_dram_parameter("bv", [Ddim], F32, isOutput=False)
    row0_ext = nc.declare_dram_parameter("row0", [1], F32, isOutput=False)
    out_ext = nc.declare_dram_parameter("out", [ROWS, Ddim], F32, isOutput=True)

    ident_d = nc.inline_tensor(np.eye(P, dtype=np.float32), name="ident_c")
    ones_d = nc.inline_tensor(np.ones((P, 8), np.float32), name="ones_c")
    # mask keeps where (m - p) + (row0 - SW*t - 128j) <= 0
    njt_np = np.broadcast_to(
        -(float(SW) * np.arange(NT)[:, None] + 128.0 * np.arange(JT)[None, :])
        .astype(np.float32).reshape(1, NT * JT), (P, NT * JT)).copy()
    njt_d = nc.inline_tensor(njt_np, name="njt_c")
    nSWt_d = nc.inline_tensor(
        np.broadcast_to((-float(SW) * np.arange(NT, dtype=np.float32))[None, :], (P, NT)).copy(),
        name="nswt_c")

    with tile.TileContext(nc) as tc:
        with tc.tile_pool(name="const", bufs=1) as constp, \
             tc.tile_pool(name="persist", bufs=1) as persist, \
             tc.tile_pool(name="dram", bufs=1, space="DRAM") as dram:
            ident = constp.tile([P, P], F32)
            nc.scalar.dma_start(out=ident[:], in_=ident_d.ap())
            ones_f = constp.tile([P, 8], F32)
            nc.scalar.dma_start(out=ones_f[:], in_=ones_d.ap())
            ones8 = constp.tile([P, 8], BF16)
            nc.vector.tensor_copy(ones8[:], ones_f[:])
            bvb = constp.tile([P, Ddim], F32)
            nc.scalar.dma_start(out=bvb[:], in_=bv_ext[:].partition_broadcast(P))
            bqs = constp.tile([P, AO], F32)
            nc.scalar.dma_start(out=bqs[:], in_=bq_ext[:].rearrange("(ao p) -> p ao", p=P))
            bks = constp.tile([P, AO], F32)
            nc.scalar.dma_start(out=bks[:], in_=bk_ext[:].rearrange("(ao p) -> p ao", p=P))
            row0b = constp.tile([P, 1], F32)
            nc.scalar.dma_start(out=row0b[:], in_=row0_ext[:].partition_broadcast(P))
            nswt = constp.tile([P, NT], F32)
            nc.scalar.dma_start(out=nswt[:], in_=nSWt_d.ap())
            r0t = constp.tile([P, NT], F32)
            nc.vector.tensor_scalar(r0t[:], nswt[:], row0b[:], None, OP.add)

            QT = persist.tile([P, IO, ROWS], BF16)
            KW = AO * ROWS               # flat K width per partition
            VW = ZB * Ddim               # flat V width per partition
            kt_bd = dram.tile([P, KW], BF16)
            v_bds = [dram.tile([P, VW // VH], BF16, name=f"v_bd{vh}") for vh in range(VH)]
            kt_gd = dram.tile([NCORES, P, KW], BF16)
            v_gds = [dram.tile([NCORES, P, VW // VH], BF16, name=f"v_gd{vh}") for vh in range(VH)]

            # ------- Phase 1+2: projections of own shards; K/V AllGathered -------
            with tc.tile_pool(name="inp", bufs=1) as inp, \
                 tc.tile_pool(name="wst", bufs=3) as wst, \
                 tc.tile_pool(name="wkv", bufs=1) as wp, \
                 tc.tile_pool(name="zp", bufs=1) as zp, \
                 tc.tile_pool(name="tpp", bufs=2, space="PSUM") as tpp, \
                 tc.tile_pool(name="pp", bufs=2, space="PSUM") as pp:
                wmup = wst.tile([P, 512], BF16, tag="wm", name="wmup")
                nc.vector.memset(wmup[:], 0.0)
                wpsum = tpp.tile([P, 512], F32, tag="wm", name="wpsum", bufs=1)
                for i in range(56):
                    nc.tensor.matmul(wpsum[:], wmup[:, 0:128], wmup[:], start=True, stop=True)
                zsb = inp.tile([P, ZB, Ddim], F32)
                nc.sync.dma_start(out=zsb[:], in_=z_ext[:])
                xsb = inp.tile([P, MB, Ddim], F32)
                nc.gpsimd.dma_start(out=xsb[:], in_=x_ext[:])
                wk = wp.tile([P, IO, Ddim], BF16)
                wv = wp.tile([P, IO, Ddim], BF16)
                wq = wp.tile([P, IO, Ddim], BF16)
                for io in range(IO):
                    ws = wst.tile([P, Ddim], F32, tag="ws", name=f"ws_k_{io}")
                    nc.scalar.dma_start(out=ws[:], in_=wk_ext[io * P:(io + 1) * P, :])
                    nc.vector.tensor_copy(wk[:, io, :], ws[:])
                zT = zp.tile([P, IO, ROWS], BF16)
                for io in range(IO):
                    for nb in range(ZB):
                        tp = tpp.tile([P, P], F32, tag="tp", name=f"tp_{nb}_{io}")
                        nc.tensor.transpose(tp[:], zsb[:, nb, io * P:(io + 1) * P], ident[:])
                        nc.vector.tensor_copy(zT[:, io, nb * P:(nb + 1) * P], tp[:])

                KTs = persist.tile([P, AO, ROWS], BF16)
                for ao in range(AO):
                    kp = pp.tile([P, ROWS], F32, tag="kp", name=f"kp_{ao}")
                    for io in range(IO):
                        nc.tensor.matmul(kp[:], wk[:, io, ao * P:(ao + 1) * P], zT[:, io, :],
                                         start=(io == 0), stop=(io == IO - 1))
                    nc.vector.tensor_scalar(KTs[:, ao, :], kp[:], bks[:, ao:ao + 1], None, OP.add)
                nc.sync.dma_start(out=kt_bd[:], in_=KTs[:])
                nc.gpsimd.collective_compute(
                    "AllGather", OP.bypass, replica_groups=[list(range(NCORES))],
                    ins=[kt_bd[:].opt()], outs=[kt_gd[:].opt()])

                # wv/wq staged after K so their casts stay off the K critical path
                for wi, (eng, wtile, wext) in enumerate((
                        (nc.scalar, wv, wv_ext), (nc.gpsimd, wq, wq_ext))):
                    for io in range(IO):
                        ws = wst.tile([P, Ddim], F32, tag="ws", name=f"ws_{wi}_{io}")
                        eng.dma_start(out=ws[:], in_=wext[io * P:(io + 1) * P, :])
                        nc.vector.tensor_copy(wtile[:, io, :], ws[:])

                # Q^T projection (overlaps the K AllGather)
                xT = zp.tile([P, IO, ROWS], BF16)
                for io in range(IO):
                    for mb in range(MB):
                        tq = tpp.tile([P, P], F32, tag="tp", name=f"tq_{mb}_{io}")
                        nc.tensor.transpose(tq[:], xsb[:, mb, io * P:(io + 1) * P], ident[:])
                        nc.vector.tensor_copy(xT[:, io, mb * P:(mb + 1) * P], tq[:])
                for ao in range(AO):
                    qp = pp.tile([P, ROWS], F32, tag="kp", name=f"qp_{ao}")
                    for io in range(IO):
                        nc.tensor.matmul(qp[:], wq[:, io, ao * P:(ao + 1) * P], xT[:, io, :],
                                         start=(io == 0), stop=(io == IO - 1))
                    # fold the softmax 1/sqrt(D) into Q^T
                    nc.vector.tensor_scalar(QT[:, ao, :], qp[:], bqs[:, ao:ao + 1], float(scale),
                                            OP.add, OP.mult)

                Vs = persist.tile([P, VH, ZB, 512], BF16)
                for nb in range(ZB):
                    vp = pp.tile([P, Ddim], F32, tag="vp", name=f"vp_{nb}", bufs=1)
                    for io in range(IO):
                        for vh in range(VH):
                            nc.tensor.matmul(vp[:, vh * 512:(vh + 1) * 512],
                                             zT[:, io, nb * P:(nb + 1) * P],
                                             wv[:, io, vh * 512:(vh + 1) * 512],
                                             start=(io == 0), stop=(io == IO - 1))
                    for vh in range(VH):
                        nc.vector.tensor_tensor(Vs[:, vh, nb, :], vp[:, vh * 512:(vh + 1) * 512],
                                                bvb[:, vh * 512:(vh + 1) * 512], OP.add)
                for vh in range(VH):
                    nc.sync.dma_start(out=v_bds[vh][:], in_=Vs[:, vh])
                    nc.gpsimd.collective_compute(
                        "AllGather", OP.bypass, replica_groups=[list(range(NCORES))],
                        ins=[v_bds[vh][:].opt()], outs=[v_gds[vh][:].opt()])

            # ---------------- Phase 3: attention ----------------
            acc = persist.tile([P, MB, Ddim], F32)       # PV accumulator (SBUF)
            with tc.tile_pool(name="ktp", bufs=2) as ktp, \
                 tc.tile_pool(name="vtp", bufs=3) as vtp, \
                 tc.tile_pool(name="esp", bufs=8) as esp, \
                 tc.tile_pool(name="recp", bufs=1) as recp:
                # nq[p, t] = 1.0 where tile t is NOT this core's own shard
                nq = constp.tile([P, NT], F32)
                nc.vector.tensor_scalar(nq[:], r0t[:], 0.0, None, OP.not_equal)
                # precompute all masks up front (hides under the AllGather):
                # mk_all[t] keeps where (m-p) + (row0 - SW*t - 128j) <= 0, t != own
                mk_all = persist.tile([P, NT, JT * ROWS], BF16)
                mk_loc = persist.tile([P, JT * ROWS], BF16)
                with tc.tile_pool(name="iop", bufs=1) as iop:
                    iota1 = iop.tile([P, ROWS], F32)
                    nc.gpsimd.iota(iota1[:], pattern=[[1, ROWS]], base=0,
                                   channel_multiplier=-1,
                                   allow_small_or_imprecise_dtypes=True)
                    njt = iop.tile([P, NT * JT], F32)
                    nc.sync.dma_start(out=njt[:], in_=njt_d.ap())
                    r0tj = iop.tile([P, NT * JT], F32)
                    nc.vector.tensor_scalar(r0tj[:], njt[:], row0b[:], None, OP.add)
                    for j in range(JT):
                        nc.vector.tensor_scalar(mk_loc[:, j * ROWS:(j + 1) * ROWS], iota1[:],
                                                float(-128 * j), 0.0, OP.add, OP.is_le)
                    for t in range(NT):
                        for j in range(JT):
                            tj = t * JT + j
                            nc.vector.tensor_scalar(mk_all[:, t, j * ROWS:(j + 1) * ROWS],
                                                    iota1[:], r0tj[:, tj:tj + 1], 0.0,
                                                    OP.add, OP.is_le)
                        nc.vector.tensor_scalar(mk_all[:, t, :], mk_all[:, t, :],
                                                nq[:, t:t + 1], None, OP.mult)

                es_list = []
                lacc = persist.tile([P, MB, 8], F32)

                def attn_pv(tag, es_grp, v_grp, vh, init, pool, lpool):
                    # one accumulation group per m-chunk, spanning the whole
                    # group of tiles; tile-major loop so each vtt is consumed
                    # once and released
                    pvs = [pool.tile([P, 512], F32, tag=f"pvq{g % 2}",
                                     name=f"pv{g}_{tag}") for g in range(MB)]
                    np_ = len(es_grp)
                    for ti, (es, v_src) in enumerate(zip(es_grp, v_grp)):
                        for j in range(JT):
                            for h in range(HLF):
                                for mc in range(2):
                                    m0 = h * 256 + mc * P
                                    nc.tensor.matmul(pvs[2 * h + mc][:],
                                                     es[:, j, m0:m0 + P], v_src[:, j, :],
                                                     start=(ti == 0 and j == 0),
                                                     stop=(ti == np_ - 1 and j == JT - 1))
                    for gmc in range(MB):
                        vsl = slice(vh * 512, (vh + 1) * 512)
                        if init:
                            nc.vector.tensor_copy(acc[:, gmc, vsl], pvs[gmc][:])
                        else:
                            nc.vector.tensor_tensor(acc[:, gmc, vsl], acc[:, gmc, vsl],
                                                    pvs[gmc][:], OP.add)

                def calc_l(tag, es_grp, init, lpool):
                    # row-sums need only es: runs in the S window, off the PV tail
                    for h in range(HLF):
                        lts = [lpool.tile([P, 8], F32, tag=f"lt{mc}",
                                          name=f"lt{mc}_{tag}_{h}") for mc in range(2)]
                        np_ = len(es_grp)
                        for ti, es in enumerate(es_grp):
                            for j in range(JT):
                                for mc in range(2):
                                    m0 = h * 256 + mc * P
                                    nc.tensor.matmul(lts[mc][:], es[:, j, m0:m0 + P],
                                                     ones8[:],
                                                     start=(ti == 0 and j == 0),
                                                     stop=(ti == np_ - 1 and j == JT - 1))
                        for mc in range(2):
                            gmc = 2 * h + mc
                            if init:
                                nc.vector.tensor_copy(lacc[:, gmc, :], lts[mc][:])
                            else:
                                nc.vector.tensor_tensor(lacc[:, gmc, :], lacc[:, gmc, :],
                                                        lts[mc][:], OP.add)

                with tc.tile_pool(name="spp", bufs=2, space="PSUM") as spp, \
                     tc.tile_pool(name="lpp", bufs=1, space="PSUM") as lpp, \
                     tc.tile_pool(name="pvg", bufs=2, space="PSUM") as pvg:

                    def attn_s(tag, kt_src, mk_ap, es_tag="es"):
                        es = esp.tile([P, JT, ROWS], BF16, tag=es_tag, name=f"es_{tag}")
                        for j in range(JT):
                            sp = spp.tile([P, ROWS], F32, tag="sp", name=f"sp_{tag}_{j}")
                            for io in range(IO):
                                nc.tensor.matmul(sp[:], kt_src[:, io, j * P:(j + 1) * P],
                                                 QT[:, io, :], start=(io == 0),
                                                 stop=(io == IO - 1))
                            nc.scalar.activation(es[:, j, :], sp[:], AF.Exp)
                        nc.vector.tensor_tensor(es[:].rearrange("p j m -> p (j m)"),
                                                es[:].rearrange("p j m -> p (j m)"),
                                                mk_ap, OP.mult)
                        return es

                    # local pre-pass on this core's own shard - overlaps the CCs
                    es_l = attn_s("loc", KTs, mk_loc[:], es_tag="esl")
                    calc_l("loc", [es_l], init=True, lpool=lpp)
                    for vh in range(VH):
                        attn_pv(f"loc{vh}", [es_l], [Vs[:, vh]], vh, init=True,
                                pool=pvg, lpool=lpp)

                    # S pass for all gathered key tiles (overlaps the V AllGathers)
                    dmae = (nc.sync, nc.scalar, nc.gpsimd)
                    for t in range(NT):
                        ktt = ktp.tile([P, IO, SW], BF16, tag="ktt", name=f"ktt_{t}")
                        dmae[t % 3].dma_start(out=ktt[:], in_=kt_gd[t])
                        es_list.append(attn_s(f"g{t}", ktt, mk_all[:, t, :]))
                        if t % 2 == 1:
                            calc_l(f"l{t}", [es_list[t - 1], es_list[t]], init=False,
                                   lpool=lpp)

                    # PV passes per value-half over tile quads; vh0 (plus all
                    # row-sums) hides under the second V AllGather
                    for vh in range(VH):
                        vgrp = []
                        for t in range(NT):
                            vtt = vtp.tile([P, JT, 512], BF16, tag="vtt",
                                           name=f"vtt_{vh}_{t}")
                            dmae[t % 2].dma_start(out=vtt[:], in_=v_gds[vh][t])
                            vgrp.append(vtt)
                        attn_pv(f"p{vh}", es_list, vgrp, vh, init=False,
                                pool=pvg, lpool=lpp)

                # normalize and write out per chunk (overlaps the PV tail)
                oview = out_ext[:].rearrange("(mb p) v -> p mb v", p=P)
                for gmc in range(MB):
                    rec = recp.tile([P, 1], F32, tag=f"rec{gmc}", name=f"rec_{gmc}")
                    nc.vector.reciprocal(rec[:], lacc[:, gmc, 0:1])
                    nc.vector.tensor_scalar(acc[:, gmc, :], acc[:, gmc, :], rec[:],
                                            None, OP.mult)
                    nc.sync.dma_start(out=oview[:, gmc, :], in_=acc[:, gmc, :])
    nc.compile()
    return nc


_GRAPH_CACHE = {}


def _get_graph(Ldim=L, Ddim=D):
    key = (Ldim, Ddim)
    if key not in _GRAPH_CACHE:
        _GRAPH_CACHE[key] = build_graph(Ldim, Ddim)
    return _GRAPH_CACHE[key]


def kernel(x, z, Wq, bq, Wk, bk, Wv, bv):
    x = np.ascontiguousarray(np.asarray(x, dtype=np.float32))
    z = np.ascontiguousarray(np.asarray(z, dtype=np.float32))
    Ldim, Ddim = x.shape
    NPART = P
    nc = _get_graph(Ldim, Ddim)
    ROWS = Ldim // NCORES
    common = {
        "Wq": np.ascontiguousarray(np.asarray(Wq, np.float32)),
        "bq": np.ascontiguousarray(np.asarray(bq, np.float32)),
        "Wk": np.ascontiguousarray(np.asarray(Wk, np.float32)),
        "bk": np.ascontiguousarray(np.asarray(bk, np.float32)),
        "Wv": np.ascontiguousarray(np.asarray(Wv, np.float32)),
        "bv": np.ascontiguousarray(np.asarray(bv, np.float32)),
    }
    in_maps = []
    for c in range(NCORES):
        m = dict(common)
        xc = x[ROWS * c:ROWS * (c + 1)]
        zc = z[ROWS * c:ROWS * (c + 1)]
        m["x"] = np.ascontiguousarray(
            xc.reshape(ROWS // NPART, NPART, Ddim).transpose(1, 0, 2))
        m["z"] = np.ascontiguousarray(
            zc.reshape(ROWS // NPART, NPART, Ddim).transpose(1, 0, 2))
        m["row0"] = np.array([ROWS * c], dtype=np.float32)
        in_maps.append(m)
    try:
        res = run_bass_kernel_spmd(nc, in_maps, core_ids=list(range(NCORES)))
    except Exception:
        # transient NRT device hiccups have been observed; one retry
        res = run_bass_kernel_spmd(nc, in_maps, core_ids=list(range(NCORES)))
    out = np.empty((Ldim, Ddim), dtype=np.float32)
    for c in range(NCORES):
        out[ROWS * c:ROWS * (c + 1)] = res.results[c]["out"]
    return out



# revision 2
# speedup vs baseline: 1.0603x; 1.0603x over previous
"""Distributed Trainium2 Bass kernel: masked (upper-triangular) attention.

reference (L=4096, D=1024, fp32):
    Q = x @ Wq + bq ; K = z @ Wk + bk ; V = z @ Wv + bv
    S = Q @ K.T ; S[row > col] = -inf
    out = softmax(S / sqrt(D)) @ V

Strategy (8 NeuronCores, one TRN2 chip, SPMD):
  - Sequence parallel on query rows: core c owns rows [512c, 512c+512).
  - K/V projection sharded over z rows (512/core), AllGathered in bf16
    (K stored transposed [D, L] blocked by shard, V natural [L, D]).
  - Attention computed as S^T tiles (keys on partitions) so the P^T needed by
    the PV matmul comes straight out of the softmax with no transposes.
  - Softmax without max-subtraction (scores here are O(1), exp can't overflow
    in fp32); mask applied multiplicatively after exp, built at runtime from
    an iota constant + a per-core row0 scalar input, keeping one graph valid
    for all cores (SPMD - no per-core control flow).
  - Matmuls in bf16 with fp32 PSUM accumulation (end-to-end rel err ~3e-3).
"""

import math

import numpy as np

import concourse.mybir as mybir
import concourse.tile as tile
from concourse import bacc
from concourse.bass_utils import run_bass_kernel_spmd

F32 = mybir.dt.float32
BF16 = mybir.dt.bfloat16
AF = mybir.ActivationFunctionType
OP = mybir.AluOpType
P = 128
NCORES = 8

L = 4096
D = 1024


def build_graph(Ldim=L, Ddim=D):
    nc = bacc.Bacc("TRN2", target_bir_lowering=False, debug=False, num_devices=NCORES)
    ROWS = Ldim // NCORES        # query rows per core
    MB = ROWS // P               # 128-row m-chunks per core (4)
    ZB = ROWS // P               # z-shard 128-row blocks (4)
    SW = ROWS                    # key-tile width == z-shard width (512)
    JT = SW // P                 # 128-row subtiles per key tile (4)
    NT = NCORES                  # one key tile per shard
    IO = Ddim // P               # contraction chunks (8)
    AO = Ddim // P               # d_attn 128-blocks (8)
    VH = Ddim // 512             # 512-wide value column halves (2)
    HLF = ROWS // 256            # 256-row halves for PV psum pressure (2)
    scale = 1.0 / math.sqrt(Ddim)

    x_ext = nc.declare_dram_parameter("x", [P, ROWS // P, Ddim], F32, isOutput=False)
    z_ext = nc.declare_dram_parameter("z", [P, ROWS // P, Ddim], F32, isOutput=False)
    wq_ext = nc.declare_dram_parameter("Wq", [Ddim, Ddim], F32, isOutput=False)
    wk_ext = nc.declare_dram_parameter("Wk", [Ddim, Ddim], F32, isOutput=False)
    wv_ext = nc.declare_dram_parameter("Wv", [Ddim, Ddim], F32, isOutput=False)
    bq_ext = nc.declare_dram_parameter("bq", [Ddim], F32, isOutput=False)
    bk_ext = nc.declare_dram_parameter("bk", [Ddim], F32, isOutput=False)
    bv_ext = nc.declare_dram_parameter("bv", [Ddim], F32, isOutput=False)
    row0_ext = nc.declare_dram_parameter("row0", [1], F32, isOutput=False)
    out_ext = nc.declare_dram_parameter("out", [ROWS, Ddim], F32, isOutput=True)

    ident_d = nc.inline_tensor(np.eye(P, dtype=np.float32), name="ident_c")
    ones_d = nc.inline_tensor(np.ones((P, 8), np.float32), name="ones_c")
    # mask keeps where (m - p) + (row0 - SW*t - 128j) <= 0
    njt_np = np.broadcast_to(
        -(float(SW) * np.arange(NT)[:, None] + 128.0 * np.arange(JT)[None, :])
        .astype(np.float32).reshape(1, NT * JT), (P, NT * JT)).copy()
    njt_d = nc.inline_tensor(njt_np, name="njt_c")
    nSWt_d = nc.inline_tensor(
        np.broadcast_to((-float(SW) * np.arange(NT, dtype=np.float32))[None, :], (P, NT)).copy(),
        name="nswt_c")

    with tile.TileContext(nc) as tc:
        with tc.tile_pool(name="const", bufs=1) as constp, \
             tc.tile_pool(name="persist", bufs=1) as persist, \
             tc.tile_pool(name="dram", bufs=1, space="DRAM") as dram:
            ident = constp.tile([P, P], F32)
            nc.scalar.dma_start(out=ident[:], in_=ident_d.ap())
            ones_f = constp.tile([P, 8], F32)
            nc.scalar.dma_start(out=ones_f[:], in_=ones_d.ap())
            ones8 = constp.tile([P, 8], BF16)
            nc.vector.tensor_copy(ones8[:], ones_f[:])
            bvb = constp.tile([P, Ddim], F32)
            nc.scalar.dma_start(out=bvb[:], in_=bv_ext[:].partition_broadcast(P))
            bqs = constp.tile([P, AO], F32)
            nc.scalar.dma_start(out=bqs[:], in_=bq_ext[:].rearrange("(ao p) -> p ao", p=P))
            bks = constp.tile([P, AO], F32)
            nc.scalar.dma_start(out=bks[:], in_=bk_ext[:].rearrange("(ao p) -> p ao", p=P))
            row0b = constp.tile([P, 1], F32)
            nc.scalar.dma_start(out=row0b[:], in_=row0_ext[:].partition_broadcast(P))
            nswt = constp.tile([P, NT], F32)
            nc.scalar.dma_start(out=nswt[:], in_=nSWt_d.ap())
            r0t = constp.tile([P, NT], F32)
            nc.vector.tensor_scalar(r0t[:], nswt[:], row0b[:], None, OP.add)

            QT = persist.tile([P, IO, ROWS], BF16)
            KW = AO * ROWS               # flat K width per partition
            VW = ZB * Ddim               # flat V width per partition
            kt_bd = dram.tile([P, KW], BF16)
            v_bds = [dram.tile([P, VW // VH], BF16, name=f"v_bd{vh}") for vh in range(VH)]
            kt_gd = dram.tile([NCORES, P, KW], BF16, addr_space="Shared")
            v_gds = [dram.tile([NCORES, P, VW // VH], BF16, name=f"v_gd{vh}",
                               addr_space="Shared") for vh in range(VH)]

            # ------- Phase 1+2: projections of own shards; K/V AllGathered -------
            # Ordered so the K AllGather launches as early as possible: the
            # attention S-pass over gathered tiles is gated on it.
            with tc.tile_pool(name="inp", bufs=1) as inp, \
                 tc.tile_pool(name="wst", bufs=3) as wst, \
                 tc.tile_pool(name="wkv", bufs=1) as wp, \
                 tc.tile_pool(name="zp", bufs=1) as zp, \
                 tc.tile_pool(name="tpp", bufs=2, space="PSUM") as tpp, \
                 tc.tile_pool(name="pp", bufs=2, space="PSUM") as pp:
                zsb = inp.tile([P, ZB, Ddim], F32)
                nc.sync.dma_start(out=zsb[:], in_=z_ext[:])
                xsb = inp.tile([P, MB, Ddim], F32)
                nc.sync.dma_start(out=xsb[:], in_=x_ext[:])
                wk = wp.tile([P, IO, Ddim], BF16)
                wv = wp.tile([P, IO, Ddim], BF16)
                wq = wp.tile([P, IO, Ddim], BF16)
                for io in range(IO):
                    ws = wst.tile([P, Ddim], F32, tag="ws", name=f"ws_k_{io}")
                    nc.scalar.dma_start(out=ws[:], in_=wk_ext[io * P:(io + 1) * P, :])
                    nc.vector.tensor_copy(wk[:, io, :], ws[:])
                zT = zp.tile([P, IO, ROWS], BF16)
                for io in range(IO):
                    for nb in range(ZB):
                        tp = tpp.tile([P, P], F32, tag="tp", name=f"tp_{nb}_{io}")
                        nc.tensor.transpose(tp[:], zsb[:, nb, io * P:(io + 1) * P], ident[:])
                        nc.vector.tensor_copy(zT[:, io, nb * P:(nb + 1) * P], tp[:])

                KTs = persist.tile([P, AO, ROWS], BF16)
                for ao in range(AO):
                    kp = pp.tile([P, ROWS], F32, tag="kp", name=f"kp_{ao}")
                    for io in range(IO):
                        nc.tensor.matmul(kp[:], wk[:, io, ao * P:(ao + 1) * P], zT[:, io, :],
                                         start=(io == 0), stop=(io == IO - 1))
                    nc.vector.tensor_scalar(KTs[:, ao, :], kp[:], bks[:, ao:ao + 1], None, OP.add)
                nc.sync.dma_start(out=kt_bd[:], in_=KTs[:])
                nc.gpsimd.collective_compute(
                    "AllGather", OP.bypass, replica_groups=[list(range(NCORES))],
                    ins=[kt_bd[:].opt()], outs=[kt_gd[:].opt()])

                # V next: its AllGathers queue on the CC engine right behind K
                for io in range(IO):
                    ws = wst.tile([P, Ddim], F32, tag="ws", name=f"ws_v_{io}")
                    nc.scalar.dma_start(out=ws[:], in_=wv_ext[io * P:(io + 1) * P, :])
                    nc.vector.tensor_copy(wv[:, io, :], ws[:])
                Vs = persist.tile([P, VH, ZB, 512], BF16)
                for nb in range(ZB):
                    vp = pp.tile([P, Ddim], F32, tag="vp", name=f"vp_{nb}", bufs=1)
                    for io in range(IO):
                        for vh in range(VH):
                            nc.tensor.matmul(vp[:, vh * 512:(vh + 1) * 512],
                                             zT[:, io, nb * P:(nb + 1) * P],
                                             wv[:, io, vh * 512:(vh + 1) * 512],
                                             start=(io == 0), stop=(io == IO - 1))
                    for vh in range(VH):
                        nc.vector.tensor_tensor(Vs[:, vh, nb, :], vp[:, vh * 512:(vh + 1) * 512],
                                                bvb[:, vh * 512:(vh + 1) * 512], OP.add)
                for vh in range(VH):
                    nc.sync.dma_start(out=v_bds[vh][:], in_=Vs[:, vh])
                    nc.gpsimd.collective_compute(
                        "AllGather", OP.bypass, replica_groups=[list(range(NCORES))],
                        ins=[v_bds[vh][:].opt()], outs=[v_gds[vh][:].opt()])

                # Q^T projection (overlaps the K/V AllGathers)
                for io in range(IO):
                    ws = wst.tile([P, Ddim], F32, tag="ws", name=f"ws_q_{io}")
                    nc.scalar.dma_start(out=ws[:], in_=wq_ext[io * P:(io + 1) * P, :])
                    nc.vector.tensor_copy(wq[:, io, :], ws[:])
                xT = zp.tile([P, IO, ROWS], BF16)
                for io in range(IO):
                    for mb in range(MB):
                        tq = tpp.tile([P, P], F32, tag="tp", name=f"tq_{mb}_{io}")
                        nc.tensor.transpose(tq[:], xsb[:, mb, io * P:(io + 1) * P], ident[:])
                        nc.vector.tensor_copy(xT[:, io, mb * P:(mb + 1) * P], tq[:])
                for ao in range(AO):
                    qp = pp.tile([P, ROWS], F32, tag="kp", name=f"qp_{ao}")
                    for io in range(IO):
                        nc.tensor.matmul(qp[:], wq[:, io, ao * P:(ao + 1) * P], xT[:, io, :],
                                         start=(io == 0), stop=(io == IO - 1))
                    # fold the softmax 1/sqrt(D) into Q^T
                    nc.vector.tensor_scalar(QT[:, ao, :], qp[:], bqs[:, ao:ao + 1], float(scale),
                                            OP.add, OP.mult)

            # ---------------- Phase 3: attention ----------------
            acc = persist.tile([P, MB, Ddim], F32)       # PV accumulator (SBUF)
            with tc.tile_pool(name="ktp", bufs=2) as ktp, \
                 tc.tile_pool(name="vtp", bufs=3) as vtp, \
                 tc.tile_pool(name="esp", bufs=8) as esp, \
                 tc.tile_pool(name="recp", bufs=1) as recp:
                # nq[p, t] = 1.0 where tile t is NOT this core's own shard
                nq = constp.tile([P, NT], F32)
                nc.vector.tensor_scalar(nq[:], r0t[:], 0.0, None, OP.not_equal)
                # precompute all masks up front (hides under the AllGather):
                # mk_all[t] keeps where (m-p) + (row0 - SW*t - 128j) <= 0, t != own
                mk_all = persist.tile([P, NT, JT * ROWS], BF16)
                mk_loc = persist.tile([P, JT * ROWS], BF16)
                with tc.tile_pool(name="iop", bufs=1) as iop:
                    iota1 = iop.tile([P, ROWS], F32)
                    nc.gpsimd.iota(iota1[:], pattern=[[1, ROWS]], base=0,
                                   channel_multiplier=-1,
                                   allow_small_or_imprecise_dtypes=True)
                    njt = iop.tile([P, NT * JT], F32)
                    nc.sync.dma_start(out=njt[:], in_=njt_d.ap())
                    r0tj = iop.tile([P, NT * JT], F32)
                    nc.vector.tensor_scalar(r0tj[:], njt[:], row0b[:], None, OP.add)
                    for j in range(JT):
                        nc.vector.tensor_scalar(mk_loc[:, j * ROWS:(j + 1) * ROWS], iota1[:],
                                                float(-128 * j), 0.0, OP.add, OP.is_le)
                    for t in range(NT):
                        for j in range(JT):
                            tj = t * JT + j
                            nc.vector.tensor_scalar(mk_all[:, t, j * ROWS:(j + 1) * ROWS],
                                                    iota1[:], r0tj[:, tj:tj + 1], 0.0,
                                                    OP.add, OP.is_le)
                        nc.vector.tensor_scalar(mk_all[:, t, :], mk_all[:, t, :],
                                                nq[:, t:t + 1], None, OP.mult)

                es_list = []
                lacc = persist.tile([P, MB, 8], F32)

                def attn_pv(tag, es_grp, v_grp, vh, init, pool, lpool):
                    # one accumulation group per m-chunk, spanning the whole
                    # group of tiles; tile-major loop so each vtt is consumed
                    # once and released
                    pvs = [pool.tile([P, 512], F32, tag=f"pvq{g % 2}",
                                     name=f"pv{g}_{tag}") for g in range(MB)]
                    np_ = len(es_grp)
                    for ti, (es, v_src) in enumerate(zip(es_grp, v_grp)):
                        for j in range(JT):
                            for h in range(HLF):
                                for mc in range(2):
                                    m0 = h * 256 + mc * P
                                    nc.tensor.matmul(pvs[2 * h + mc][:],
                                                     es[:, j, m0:m0 + P], v_src[:, j, :],
                                                     start=(ti == 0 and j == 0),
                                                     stop=(ti == np_ - 1 and j == JT - 1))
                    for gmc in range(MB):
                        vsl = slice(vh * 512, (vh + 1) * 512)
                        if init:
                            nc.vector.tensor_copy(acc[:, gmc, vsl], pvs[gmc][:])
                        else:
                            nc.vector.tensor_tensor(acc[:, gmc, vsl], acc[:, gmc, vsl],
                                                    pvs[gmc][:], OP.add)

                def calc_l(tag, es_grp, init, lpool):
                    # row-sums need only es: runs in the S window, off the PV tail
                    for h in range(HLF):
                        lts = [lpool.tile([P, 8], F32, tag=f"lt{mc}",
                                          name=f"lt{mc}_{tag}_{h}") for mc in range(2)]
                        np_ = len(es_grp)
                        for ti, es in enumerate(es_grp):
                            for j in range(JT):
                                for mc in range(2):
                                    m0 = h * 256 + mc * P
                                    nc.tensor.matmul(lts[mc][:], es[:, j, m0:m0 + P],
                                                     ones8[:],
                                                     start=(ti == 0 and j == 0),
                                                     stop=(ti == np_ - 1 and j == JT - 1))
                        for mc in range(2):
                            gmc = 2 * h + mc
                            if init:
                                nc.vector.tensor_copy(lacc[:, gmc, :], lts[mc][:])
                            else:
                                nc.vector.tensor_tensor(lacc[:, gmc, :], lacc[:, gmc, :],
                                                        lts[mc][:], OP.add)

                with tc.tile_pool(name="spp", bufs=2, space="PSUM") as spp, \
                     tc.tile_pool(name="lpp", bufs=1, space="PSUM") as lpp, \
                     tc.tile_pool(name="pvg", bufs=2, space="PSUM") as pvg:

                    def attn_s(tag, kt_src, mk_ap, es_tag="es"):
                        es = esp.tile([P, JT, ROWS], BF16, tag=es_tag, name=f"es_{tag}")
                        for j in range(JT):
                            sp = spp.tile([P, ROWS], F32, tag="sp", name=f"sp_{tag}_{j}")
                            for io in range(IO):
                                nc.tensor.matmul(sp[:], kt_src[:, io, j * P:(j + 1) * P],
                                                 QT[:, io, :], start=(io == 0),
                                                 stop=(io == IO - 1))
                            nc.scalar.activation(es[:, j, :], sp[:], AF.Exp)
                        nc.vector.tensor_tensor(es[:].rearrange("p j m -> p (j m)"),
                                                es[:].rearrange("p j m -> p (j m)"),
                                                mk_ap, OP.mult)
                        return es

                    # local pre-pass on this core's own shard - overlaps the CCs
                    es_l = attn_s("loc", KTs, mk_loc[:], es_tag="esl")
                    calc_l("loc", [es_l], init=True, lpool=lpp)
                    for vh in range(VH):
                        attn_pv(f"loc{vh}", [es_l], [Vs[:, vh]], vh, init=True,
                                pool=pvg, lpool=lpp)

                    # S pass for all gathered key tiles (overlaps the V AllGathers)
                    dmae = (nc.sync, nc.scalar, nc.gpsimd)
                    for t in range(NT):
                        ktt = ktp.tile([P, IO, SW], BF16, tag="ktt", name=f"ktt_{t}")
                        dmae[t % 3].dma_start(out=ktt[:], in_=kt_gd[t])
                        es_list.append(attn_s(f"g{t}", ktt, mk_all[:, t, :]))
                        if t % 2 == 1:
                            calc_l(f"l{t}", [es_list[t - 1], es_list[t]], init=False,
                                   lpool=lpp)

                    # PV passes per value-half over tile quads; vh0 (plus all
                    # row-sums) hides under the second V AllGather
                    for vh in range(VH):
                        vgrp = []
                        for t in range(NT):
                            vtt = vtp.tile([P, JT, 512], BF16, tag="vtt",
                                           name=f"vtt_{vh}_{t}")
                            dmae[t % 2].dma_start(out=vtt[:], in_=v_gds[vh][t])
                            vgrp.append(vtt)
                        attn_pv(f"p{vh}", es_list, vgrp, vh, init=False,
                                pool=pvg, lpool=lpp)

                # normalize and write out per chunk (overlaps the PV tail)
                oview = out_ext[:].rearrange("(mb p) v -> p mb v", p=P)
                for gmc in range(MB):
                    rec = recp.tile([P, 1], F32, tag=f"rec{gmc}", name=f"rec_{gmc}")
                    nc.vector.reciprocal(rec[:], lacc[:, gmc, 0:1])
                    nc.vector.tensor_scalar(acc[:, gmc, :], acc[:, gmc, :], rec[:],
                                            None, OP.mult)
                    nc.sync.dma_start(out=oview[:, gmc, :], in_=acc[:, gmc, :])
    nc.compile()
    return nc


_GRAPH_CACHE = {}


def _get_graph(Ldim=L, Ddim=D):
    key = (Ldim, Ddim)
    if key not in _GRAPH_CACHE:
        _GRAPH_CACHE[key] = build_graph(Ldim, Ddim)
    return _GRAPH_CACHE[key]


def kernel(x, z, Wq, bq, Wk, bk, Wv, bv):
    x = np.ascontiguousarray(np.asarray(x, dtype=np.float32))
    z = np.ascontiguousarray(np.asarray(z, dtype=np.float32))
    Ldim, Ddim = x.shape
    NPART = P
    nc = _get_graph(Ldim, Ddim)
    ROWS = Ldim // NCORES
    common = {
        "Wq": np.ascontiguousarray(np.asarray(Wq, np.float32)),
        "bq": np.ascontiguousarray(np.asarray(bq, np.float32)),
        "Wk": np.ascontiguousarray(np.asarray(Wk, np.float32)),
        "bk": np.ascontiguousarray(np.asarray(bk, np.float32)),
        "Wv": np.ascontiguousarray(np.asarray(Wv, np.float32)),
        "bv": np.ascontiguousarray(np.asarray(bv, np.float32)),
    }
    in_maps = []
    for c in range(NCORES):
        m = dict(common)
        xc = x[ROWS * c:ROWS * (c + 1)]
        zc = z[ROWS * c:ROWS * (c + 1)]
        m["x"] = np.ascontiguousarray(
            xc.reshape(ROWS // NPART, NPART, Ddim).transpose(1, 0, 2))
        m["z"] = np.ascontiguousarray(
            zc.reshape(ROWS // NPART, NPART, Ddim).transpose(1, 0, 2))
        m["row0"] = np.array([ROWS * c], dtype=np.float32)
        in_maps.append(m)
    try:
        res = run_bass_kernel_spmd(nc, in_maps, core_ids=list(range(NCORES)))
    except Exception:
        # transient NRT device hiccups have been observed; one retry
        res = run_bass_kernel_spmd(nc, in_maps, core_ids=list(range(NCORES)))
    out = np.empty((Ldim, Ddim), dtype=np.float32)
    for c in range(NCORES):
        out[ROWS * c:ROWS * (c + 1)] = res.results[c]["out"]
    return out



# revision 4
# speedup vs baseline: 1.2819x; 1.2090x over previous
"""Distributed Trainium2 Bass kernel: masked (upper-triangular) attention.

reference (L=4096, D=1024, fp32):
    Q = x @ Wq + bq ; K = z @ Wk + bk ; V = z @ Wv + bv
    S = Q @ K.T ; S[row > col] = -inf
    out = softmax(S / sqrt(D)) @ V

Strategy (8 NeuronCores, one TRN2 chip, SPMD):
  - Query rows dealt round-robin: core c owns rows {r : r % 8 == c}. This
    makes the causal (keep col >= row) footprint IDENTICAL on every core:
    query chunk m (128 local rows = global rows c+8*(128m..)) attends key
    tile t (512 keys) iff 2m <= t -> a uniform static 20-unit schedule that
    skips ~44% of the S/PV work with no per-core addressing.
  - K/V projections sharded over contiguous z blocks (512/core), AllGathered
    in bf16 into Shared-address-space DRAM (K^T as [d,keys], V natural).
  - S computed in [q, k] orientation (Q^T chunk stationary, K^T tile moving
    512-wide); exp on scalar engine emits row-sums via accum_out; P^T for
    the PV matmul obtained with PE transposes of the 128x128 es chunks.
  - Only the two near-diagonal tiles per chunk need masks: two constant
    [128,512] additive (-50) masks built once from an iota + core id.
  - Matmuls in bf16 with fp32 PSUM accumulation.
"""

import math

import numpy as np

import concourse.mybir as mybir
import concourse.tile as tile
from concourse import bacc
from concourse.bass_utils import run_bass_kernel_spmd

F32 = mybir.dt.float32
BF16 = mybir.dt.bfloat16
AF = mybir.ActivationFunctionType
OP = mybir.AluOpType
P = 128
NCORES = 8

L = 4096
D = 1024


def build_graph(Ldim=L, Ddim=D):
    nc = bacc.Bacc("TRN2", target_bir_lowering=False, debug=False, num_devices=NCORES)
    ROWS = Ldim // NCORES        # query rows per core
    MB = ROWS // P               # 128-row query chunks per core (4)
    ZB = ROWS // P               # z-shard 128-row blocks (4)
    SW = ROWS                    # key-tile width == z-shard width (512)
    JT = SW // P                 # 128-key subtiles per key tile (4)
    NT = NCORES                  # one key tile per shard
    IO = Ddim // P               # contraction chunks (8)
    AO = Ddim // P               # d_attn 128-blocks (8)
    VH = Ddim // 512             # 512-wide value column halves (2)
    scale = 1.0 / math.sqrt(Ddim)
    # units (t, m) with 2m <= t; unit index = UOFF[t] + m
    UCNT = [t // 2 + 1 for t in range(NT)]
    UOFF = [sum(UCNT[:t]) for t in range(NT)]
    NU = sum(UCNT)               # 20

    x_ext = nc.declare_dram_parameter("x", [P, MB, Ddim], F32, isOutput=False)
    z_ext = nc.declare_dram_parameter("z", [P, ZB, Ddim], F32, isOutput=False)
    wq_ext = nc.declare_dram_parameter("Wq", [Ddim, Ddim], F32, isOutput=False)
    wk_ext = nc.declare_dram_parameter("Wk", [Ddim, Ddim], F32, isOutput=False)
    wv_ext = nc.declare_dram_parameter("Wv", [Ddim, Ddim], F32, isOutput=False)
    bq_ext = nc.declare_dram_parameter("bq", [Ddim], F32, isOutput=False)
    bk_ext = nc.declare_dram_parameter("bk", [Ddim], F32, isOutput=False)
    bv_ext = nc.declare_dram_parameter("bv", [Ddim], F32, isOutput=False)
    cval_ext = nc.declare_dram_parameter("cval", [1], F32, isOutput=False)
    out_ext = nc.declare_dram_parameter("out", [ROWS, Ddim], F32, isOutput=True)

    ident_d = nc.inline_tensor(np.eye(P, dtype=np.float32), name="ident_c")
    identb_d = nc.inline_tensor(np.eye(P, dtype=np.float32), name="identb_c")

    with tile.TileContext(nc) as tc:
        with tc.tile_pool(name="const", bufs=1) as constp, \
             tc.tile_pool(name="persist", bufs=1) as persist, \
             tc.tile_pool(name="dram", bufs=1, space="DRAM") as dram:
            ident = constp.tile([P, P], F32)
            nc.scalar.dma_start(out=ident[:], in_=ident_d.ap())
            identf = constp.tile([P, P], F32)
            nc.scalar.dma_start(out=identf[:], in_=identb_d.ap())
            identb = constp.tile([P, P], BF16)
            nc.vector.tensor_copy(identb[:], identf[:])
            bvb = constp.tile([P, Ddim], F32)
            nc.scalar.dma_start(out=bvb[:], in_=bv_ext[:].partition_broadcast(P))
            bqs = constp.tile([P, AO], F32)
            nc.scalar.dma_start(out=bqs[:], in_=bq_ext[:].rearrange("(ao p) -> p ao", p=P))
            bks = constp.tile([P, AO], F32)
            nc.scalar.dma_start(out=bks[:], in_=bk_ext[:].rearrange("(ao p) -> p ao", p=P))
            cvb = constp.tile([P, 1], F32)
            nc.scalar.dma_start(out=cvb[:], in_=cval_ext[:].partition_broadcast(P))

            QT = persist.tile([P, AO, ROWS], BF16)
            KW = AO * ROWS               # flat K width per partition
            VW = ZB * Ddim               # flat V width per partition
            kt_bd = dram.tile([P, KW], BF16)
            v_bds = [dram.tile([P, VW // VH], BF16, name=f"v_bd{vh}") for vh in range(VH)]
            kt_gd = dram.tile([NCORES, P, KW], BF16, addr_space="Shared")
            v_gds = [dram.tile([NCORES, P, VW // VH], BF16, name=f"v_gd{vh}",
                               addr_space="Shared") for vh in range(VH)]

            # additive pre-softmax masks for the two near-diagonal tiles of
            # each query chunk: with r = c + 8i + 1024m, keys k = 512t + f:
            #   t == 2m  : keep iff f - 8i - c >= 0        (maskA)
            #   t == 2m+1: keep iff f - 8i - c + 512 >= 0  (maskB)
            maskA = persist.tile([P, SW], F32)
            maskB = persist.tile([P, SW], F32)
            with tc.tile_pool(name="iop", bufs=1) as iop:
                iof = iop.tile([P, SW], F32)
                nc.gpsimd.iota(iof[:], pattern=[[1, SW]], base=0,
                               channel_multiplier=-8,
                               allow_small_or_imprecise_dtypes=True)
                tA = iop.tile([P, SW], F32)
                nc.vector.tensor_scalar(tA[:], iof[:], cvb[:], None, OP.subtract)
                mkA = iop.tile([P, SW], F32)
                nc.vector.tensor_scalar(mkA[:], tA[:], 0.0, None, OP.is_ge)
                nc.vector.tensor_scalar(maskA[:], mkA[:], 1.0, 50.0, OP.subtract, OP.mult)
                tB = iop.tile([P, SW], F32)
                nc.vector.tensor_scalar(tB[:], tA[:], 512.0, None, OP.add)
                mkB = iop.tile([P, SW], F32)
                nc.vector.tensor_scalar(mkB[:], tB[:], 0.0, None, OP.is_ge)
                nc.vector.tensor_scalar(maskB[:], mkB[:], 1.0, 50.0, OP.subtract, OP.mult)

            # ------- Phase 1+2: projections of own shards; K/V AllGathered -------
            with tc.tile_pool(name="inp", bufs=1) as inp, \
                 tc.tile_pool(name="wst", bufs=3) as wst, \
                 tc.tile_pool(name="wkv", bufs=1) as wp, \
                 tc.tile_pool(name="zp", bufs=1) as zp, \
                 tc.tile_pool(name="tpp", bufs=2, space="PSUM") as tpp, \
                 tc.tile_pool(name="pp", bufs=2, space="PSUM") as pp:
                zsb = inp.tile([P, ZB, Ddim], F32)
                nc.sync.dma_start(out=zsb[:], in_=z_ext[:])
                xsb = inp.tile([P, MB, Ddim], F32)
                nc.sync.dma_start(out=xsb[:], in_=x_ext[:])
                wk = wp.tile([P, IO, Ddim], BF16)
                wv = wp.tile([P, IO, Ddim], BF16)
                wq = wp.tile([P, IO, Ddim], BF16)
                for io in range(IO):
                    ws = wst.tile([P, Ddim], F32, tag="ws", name=f"ws_k_{io}")
                    nc.scalar.dma_start(out=ws[:], in_=wk_ext[io * P:(io + 1) * P, :])
                    nc.vector.tensor_copy(wk[:, io, :], ws[:])
                zT = zp.tile([P, IO, ROWS], BF16)
                for io in range(IO):
                    for nb in range(ZB):
                        tp = tpp.tile([P, P], F32, tag="tp", name=f"tp_{nb}_{io}")
                        nc.tensor.transpose(tp[:], zsb[:, nb, io * P:(io + 1) * P], ident[:])
                        nc.vector.tensor_copy(zT[:, io, nb * P:(nb + 1) * P], tp[:])

                KTs = inp.tile([P, AO, ROWS], BF16)
                for ao in range(AO):
                    kp = pp.tile([P, ROWS], F32, tag="kp", name=f"kp_{ao}")
                    for io in range(IO):
                        nc.tensor.matmul(kp[:], wk[:, io, ao * P:(ao + 1) * P], zT[:, io, :],
                                         start=(io == 0), stop=(io == IO - 1))
                    nc.vector.tensor_scalar(KTs[:, ao, :], kp[:], bks[:, ao:ao + 1], None, OP.add)
                nc.sync.dma_start(out=kt_bd[:], in_=KTs[:])
                nc.gpsimd.collective_compute(
                    "AllGather", OP.bypass, replica_groups=[list(range(NCORES))],
                    ins=[kt_bd[:].opt()], outs=[kt_gd[:].opt()])

                # V next: its AllGathers queue on the CC engine right behind K
                for io in range(IO):
                    ws = wst.tile([P, Ddim], F32, tag="ws", name=f"ws_v_{io}")
                    nc.scalar.dma_start(out=ws[:], in_=wv_ext[io * P:(io + 1) * P, :])
                    nc.vector.tensor_copy(wv[:, io, :], ws[:])
                Vs = inp.tile([P, VH, ZB, 512], BF16)
                for nb in range(ZB):
                    vp = pp.tile([P, Ddim], F32, tag="vp", name=f"vp_{nb}", bufs=1)
                    for io in range(IO):
                        for vh in range(VH):
                            nc.tensor.matmul(vp[:, vh * 512:(vh + 1) * 512],
                                             zT[:, io, nb * P:(nb + 1) * P],
                                             wv[:, io, vh * 512:(vh + 1) * 512],
                                             start=(io == 0), stop=(io == IO - 1))
                    for vh in range(VH):
                        nc.vector.tensor_tensor(Vs[:, vh, nb, :], vp[:, vh * 512:(vh + 1) * 512],
                                                bvb[:, vh * 512:(vh + 1) * 512], OP.add)
                for vh in range(VH):
                    nc.sync.dma_start(out=v_bds[vh][:], in_=Vs[:, vh])
                    nc.gpsimd.collective_compute(
                        "AllGather", OP.bypass, replica_groups=[list(range(NCORES))],
                        ins=[v_bds[vh][:].opt()], outs=[v_gds[vh][:].opt()])

                # Q^T projection (overlaps the K/V AllGathers)
                for io in range(IO):
                    ws = wst.tile([P, Ddim], F32, tag="ws", name=f"ws_q_{io}")
                    nc.scalar.dma_start(out=ws[:], in_=wq_ext[io * P:(io + 1) * P, :])
                    nc.vector.tensor_copy(wq[:, io, :], ws[:])
                xT = zp.tile([P, IO, ROWS], BF16)
                for io in range(IO):
                    for mb in range(MB):
                        tq = tpp.tile([P, P], F32, tag="tp", name=f"tq_{mb}_{io}")
                        nc.tensor.transpose(tq[:], xsb[:, mb, io * P:(io + 1) * P], ident[:])
                        nc.vector.tensor_copy(xT[:, io, mb * P:(mb + 1) * P], tq[:])
                for ao in range(AO):
                    qp = pp.tile([P, ROWS], F32, tag="kp", name=f"qp_{ao}")
                    for io in range(IO):
                        nc.tensor.matmul(qp[:], wq[:, io, ao * P:(ao + 1) * P], xT[:, io, :],
                                         start=(io == 0), stop=(io == IO - 1))
                    # fold the softmax 1/sqrt(D) into Q^T
                    nc.vector.tensor_scalar(QT[:, ao, :], qp[:], bqs[:, ao:ao + 1], float(scale),
                                            OP.add, OP.mult)

            # ---------------- Phase 3: attention ----------------
            esT = persist.tile([P, NU, JT, P], BF16)     # P^T chunks for PV
            lacc = persist.tile([P, MB], F32)            # softmax denominators
            acc = persist.tile([P, MB, Ddim], F32)       # normalized output staging
            dmae = (nc.sync, nc.scalar)

            # S pass: S[q,k] = Q^T-chunk (stationary) x K^T tile (moving);
            # exp on scalar emits row-sums via accum_out; PE transposes yield
            # the P^T chunks for PV. Transposes are enqueued one unit behind
            # so the tensor engine never waits on the exp.
            with tc.tile_pool(name="ktp", bufs=3) as ktp, \
                 tc.tile_pool(name="esp", bufs=3) as esp, \
                 tc.tile_pool(name="lpps", bufs=4) as lpps, \
                 tc.tile_pool(name="spp", bufs=2, space="PSUM") as spp, \
                 tc.tile_pool(name="tp2", bufs=2, space="PSUM") as tp2:
                pend = []

                def flush_pend():
                    for (pes, pu) in pend:
                        for kc in range(JT):
                            tp = tp2.tile([P, P], BF16, tag="tp2", name=f"tp2_{pu}_{kc}")
                            nc.tensor.transpose(tp[:], pes[:, kc * P:(kc + 1) * P], identb[:])
                            nc.scalar.activation(esT[:, pu, kc, :], tp[:], AF.Copy)
                    pend.clear()

                for t in range(NT):
                    ktt = ktp.tile([P, AO, SW], BF16, tag="ktt", name=f"ktt_{t}")
                    dmae[t % 2].dma_start(out=ktt[:], in_=kt_gd[t])
                    for m in range(t // 2 + 1):
                        u = UOFF[t] + m
                        sp = spp.tile([P, SW], F32, tag="sp", name=f"sp_{u}")
                        for ao in range(AO):
                            nc.tensor.matmul(sp[:], QT[:, ao, m * P:(m + 1) * P],
                                             ktt[:, ao, :], start=(ao == 0),
                                             stop=(ao == AO - 1))
                        flush_pend()
                        if t == 2 * m:
                            nc.vector.tensor_tensor(sp[:], sp[:], maskA[:], OP.add)
                        elif t == 2 * m + 1:
                            nc.vector.tensor_tensor(sp[:], sp[:], maskB[:], OP.add)
                        es = esp.tile([P, SW], BF16, tag="es", name=f"es_{u}")
                        lp = lpps.tile([P, 1], F32, tag="lp", name=f"lp_{u}")
                        nc.scalar.activation(es[:], sp[:], AF.Exp, accum_out=lp[:])
                        if t == 2 * m:
                            nc.vector.tensor_copy(lacc[:, m:m + 1], lp[:])
                        else:
                            nc.vector.tensor_tensor(lacc[:, m:m + 1], lacc[:, m:m + 1],
                                                    lp[:], OP.add)
                        pend.append((es, u))
                flush_pend()

            # PV pass per value-half, tiles descending so the deepest chunks
            # start immediately after the S pass; psum per query chunk.
            with tc.tile_pool(name="vtp", bufs=3) as vtp, \
                 tc.tile_pool(name="recp", bufs=1) as recp, \
                 tc.tile_pool(name="pvp", bufs=1, space="PSUM") as pvp:
                rec = recp.tile([P, MB], F32)
                nc.vector.reciprocal(rec[:], lacc[:])
                for vh in range(VH):
                    pvs = [pvp.tile([P, 512], F32, tag=f"pv{m}", name=f"pv{vh}_{m}")
                           for m in range(MB)]
                    for t in range(NT - 1, -1, -1):
                        vtt = vtp.tile([P, JT, 512], BF16, tag="vtt", name=f"vtt_{vh}_{t}")
                        dmae[t % 2].dma_start(out=vtt[:], in_=v_gds[vh][t])
                        for m in range(t // 2 + 1):
                            u = UOFF[t] + m
                            for kc in range(JT):
                                nc.tensor.matmul(pvs[m][:], esT[:, u, kc, :],
                                                 vtt[:, kc, :],
                                                 start=(t == NT - 1 and kc == 0),
                                                 stop=(t == 2 * m and kc == JT - 1))
                    for m in range(MB):
                        nc.vector.tensor_scalar(acc[:, m, vh * 512:(vh + 1) * 512],
                                                pvs[m][:], rec[:, m:m + 1], None, OP.mult)

            # write out per chunk
            oview = out_ext[:].rearrange("(mb p) v -> p mb v", p=P)
            for m in range(MB):
                nc.sync.dma_start(out=oview[:, m, :], in_=acc[:, m, :])
    nc.compile()
    return nc


_GRAPH_CACHE = {}


def _get_graph(Ldim=L, Ddim=D):
    key = (Ldim, Ddim)
    if key not in _GRAPH_CACHE:
        _GRAPH_CACHE[key] = build_graph(Ldim, Ddim)
    return _GRAPH_CACHE[key]


def kernel(x, z, Wq, bq, Wk, bk, Wv, bv):
    x = np.ascontiguousarray(np.asarray(x, dtype=np.float32))
    z = np.ascontiguousarray(np.asarray(z, dtype=np.float32))
    Ldim, Ddim = x.shape
    NPART = P
    nc = _get_graph(Ldim, Ddim)
    ROWS = Ldim // NCORES
    common = {
        "Wq": np.ascontiguousarray(np.asarray(Wq, np.float32)),
        "bq": np.ascontiguousarray(np.asarray(bq, np.float32)),
        "Wk": np.ascontiguousarray(np.asarray(Wk, np.float32)),
        "bk": np.ascontiguousarray(np.asarray(bk, np.float32)),
        "Wv": np.ascontiguousarray(np.asarray(Wv, np.float32)),
        "bv": np.ascontiguousarray(np.asarray(bv, np.float32)),
    }
    in_maps = []
    for c in range(NCORES):
        m = dict(common)
        xc = x[c::NCORES]                      # interleaved query rows
        zc = z[ROWS * c:ROWS * (c + 1)]        # contiguous key rows
        m["x"] = np.ascontiguousarray(
            xc.reshape(ROWS // NPART, NPART, Ddim).transpose(1, 0, 2))
        m["z"] = np.ascontiguousarray(
            zc.reshape(ROWS // NPART, NPART, Ddim).transpose(1, 0, 2))
        m["cval"] = np.array([c], dtype=np.float32)
        in_maps.append(m)
    try:
        res = run_bass_kernel_spmd(nc, in_maps, core_ids=list(range(NCORES)))
    except Exception:
        # transient NRT device hiccups have been observed; one retry
        res = run_bass_kernel_spmd(nc, in_maps, core_ids=list(range(NCORES)))
    out = np.empty((Ldim, Ddim), dtype=np.float32)
    for c in range(NCORES):
        out[c::NCORES] = res.results[c]["out"]
    return out


# revision 12
# speedup vs baseline: 1.3020x; 1.0156x over previous
"""Distributed Trainium2 Bass kernel: masked (upper-triangular) attention.

reference (L=4096, D=1024, fp32):
    Q = x @ Wq + bq ; K = z @ Wk + bk ; V = z @ Wv + bv
    S = Q @ K.T ; S[row > col] = -inf
    out = softmax(S / sqrt(D)) @ V

Strategy (8 NeuronCores, one TRN2 chip, SPMD):
  - Query rows dealt round-robin: core c owns rows {r : r % 8 == c}. This
    makes the causal (keep col >= row) footprint IDENTICAL on every core:
    query chunk m (128 local rows = global rows c+8*(128m..)) attends key
    tile t (512 keys) iff 2m <= t -> a uniform static 20-unit schedule that
    skips ~44% of the S/PV work with no per-core addressing.
  - K/V projections sharded over contiguous z blocks (512/core), AllGathered
    in bf16 into Shared-address-space DRAM (K^T as [d,keys], V natural).
  - S computed in [q, k] orientation (Q^T chunk stationary, K^T tile moving
    512-wide); exp on scalar engine emits row-sums via accum_out; P^T for
    the PV matmul obtained with PE transposes of the 128x128 es chunks.
  - Only the two near-diagonal tiles per chunk need masks: two constant
    [128,512] additive (-50) masks built once from an iota + core id.
  - Matmuls in bf16 with fp32 PSUM accumulation.
"""

import math

import ml_dtypes
import numpy as np

BF16_NP = ml_dtypes.bfloat16

import concourse.mybir as mybir
import concourse.tile as tile
from concourse import bacc
from concourse.bass_utils import run_bass_kernel_spmd

F32 = mybir.dt.float32
BF16 = mybir.dt.bfloat16
AF = mybir.ActivationFunctionType
OP = mybir.AluOpType
P = 128
NCORES = 8

L = 4096
D = 1024


def build_graph(Ldim=L, Ddim=D):
    nc = bacc.Bacc("TRN2", target_bir_lowering=False, debug=False, num_devices=NCORES)
    ROWS = Ldim // NCORES        # query rows per core
    MB = ROWS // P               # 128-row query chunks per core (4)
    ZB = ROWS // P               # z-shard 128-row blocks (4)
    SW = ROWS                    # key-tile width == z-shard width (512)
    JT = SW // P                 # 128-key subtiles per key tile (4)
    NT = NCORES                  # one key tile per shard
    IO = Ddim // P               # contraction chunks (8)
    AO = Ddim // P               # d_attn 128-blocks (8)
    VH = Ddim // 512             # 512-wide value column halves (2)
    scale = 1.0 / math.sqrt(Ddim)
    # units (t, m) with 2m <= t; unit index = UOFF[t] + m
    UCNT = [t // 2 + 1 for t in range(NT)]
    UOFF = [sum(UCNT[:t]) for t in range(NT)]
    NU = sum(UCNT)               # 20

    x_ext = nc.declare_dram_parameter("x", [P, MB, Ddim], BF16, isOutput=False)
    z_ext = nc.declare_dram_parameter("z", [P, ZB, Ddim], BF16, isOutput=False)
    wq_ext = nc.declare_dram_parameter("Wq", [Ddim, Ddim], BF16, isOutput=False)
    wk_ext = nc.declare_dram_parameter("Wk", [Ddim, Ddim], BF16, isOutput=False)
    wv_ext = nc.declare_dram_parameter("Wv", [Ddim, Ddim], BF16, isOutput=False)
    bq_ext = nc.declare_dram_parameter("bq", [Ddim], F32, isOutput=False)
    bk_ext = nc.declare_dram_parameter("bk", [Ddim], F32, isOutput=False)
    bv_ext = nc.declare_dram_parameter("bv", [Ddim], F32, isOutput=False)
    cval_ext = nc.declare_dram_parameter("cval", [1], F32, isOutput=False)
    out_ext = nc.declare_dram_parameter("out", [ROWS, Ddim], F32, isOutput=True)

    ident_d = nc.inline_tensor(np.eye(P, dtype=np.float32), name="ident_c")
    identb_d = nc.inline_tensor(np.eye(P, dtype=np.float32), name="identb_c")

    with tile.TileContext(nc) as tc:
        with tc.tile_pool(name="const", bufs=1) as constp, \
             tc.tile_pool(name="persist", bufs=1) as persist, \
             tc.tile_pool(name="dram", bufs=1, space="DRAM") as dram:
            identf = constp.tile([P, P], F32)
            nc.scalar.dma_start(out=identf[:], in_=identb_d.ap())
            identb = constp.tile([P, P], BF16)
            nc.vector.tensor_copy(identb[:], identf[:])
            bvb = constp.tile([P, Ddim], F32)
            nc.scalar.dma_start(out=bvb[:], in_=bv_ext[:].partition_broadcast(P))
            bqs = constp.tile([P, AO], F32)
            nc.scalar.dma_start(out=bqs[:], in_=bq_ext[:].rearrange("(ao p) -> p ao", p=P))
            bks = constp.tile([P, AO], F32)
            nc.scalar.dma_start(out=bks[:], in_=bk_ext[:].rearrange("(ao p) -> p ao", p=P))
            cvb = constp.tile([P, 1], F32)
            nc.scalar.dma_start(out=cvb[:], in_=cval_ext[:].partition_broadcast(P))

            QT = persist.tile([P, AO, ROWS], BF16)
            KW = AO * ROWS               # flat K width per partition
            VW = ZB * Ddim               # flat V width per partition
            kt_bd = dram.tile([P, KW], BF16)
            v_bd = dram.tile([P, VW], BF16)
            kt_gd = dram.tile([NCORES, P, KW], BF16, addr_space="Shared")
            v_gd = dram.tile([NCORES, P, VW], BF16, addr_space="Shared")

            # additive pre-softmax masks for the two near-diagonal tiles of
            # each query chunk: with r = c + 8i + 1024m, keys k = 512t + f:
            #   t == 2m  : keep iff f - 8i - c >= 0        (maskA)
            #   t == 2m+1: keep iff f - 8i - c + 512 >= 0  (maskB)
            maskA = persist.tile([P, SW], F32)
            maskB = persist.tile([P, SW], F32)
            with tc.tile_pool(name="iop", bufs=1) as iop:
                iof = iop.tile([P, SW], F32)
                nc.gpsimd.iota(iof[:], pattern=[[1, SW]], base=0,
                               channel_multiplier=-8,
                               allow_small_or_imprecise_dtypes=True)
                tA = iop.tile([P, SW], F32)
                nc.vector.tensor_scalar(tA[:], iof[:], cvb[:], None, OP.subtract)
                mkA = iop.tile([P, SW], F32)
                nc.vector.tensor_scalar(mkA[:], tA[:], 0.0, None, OP.is_ge)
                nc.vector.tensor_scalar(maskA[:], mkA[:], 1.0, 50.0, OP.subtract, OP.mult)
                tB = iop.tile([P, SW], F32)
                nc.vector.tensor_scalar(tB[:], tA[:], 512.0, None, OP.add)
                mkB = iop.tile([P, SW], F32)
                nc.vector.tensor_scalar(mkB[:], tB[:], 0.0, None, OP.is_ge)
                nc.vector.tensor_scalar(maskB[:], mkB[:], 1.0, 50.0, OP.subtract, OP.mult)

            # ------- Phase 1+2: projections of own shards; K/V AllGathered -------
            # Inputs arrive pre-cast to bf16 from the host: weights DMA straight
            # into their SBUF tiles (no staging/cast), transposes run in bf16.
            with tc.tile_pool(name="inp", bufs=1) as inp, \
                 tc.tile_pool(name="wkv", bufs=1) as wp, \
                 tc.tile_pool(name="zp", bufs=1) as zp, \
                 tc.tile_pool(name="tpp", bufs=2, space="PSUM") as tpp, \
                 tc.tile_pool(name="pp", bufs=2, space="PSUM") as pp:
                zsb = inp.tile([P, ZB, Ddim], BF16)
                nc.sync.dma_start(out=zsb[:], in_=z_ext[:])
                xsb = inp.tile([P, MB, Ddim], BF16)
                nc.sync.dma_start(out=xsb[:], in_=x_ext[:])
                wk = wp.tile([P, IO, Ddim], BF16)
                wv = wp.tile([P, IO, Ddim], BF16)
                wq = wp.tile([P, IO, Ddim], BF16)
                nc.scalar.dma_start(out=wk[:], in_=wk_ext[:].rearrange("(io p) d -> p io d", p=P))
                zT = zp.tile([P, IO, ROWS], BF16)
                for io in range(IO):
                    for nb in range(ZB):
                        tp = tpp.tile([P, P], BF16, tag="tp", name=f"tp_{nb}_{io}")
                        nc.tensor.transpose(tp[:], zsb[:, nb, io * P:(io + 1) * P], identb[:])
                        nc.vector.tensor_copy(zT[:, io, nb * P:(nb + 1) * P], tp[:])

                KTs = inp.tile([P, AO, ROWS], BF16)
                for ao in range(AO):
                    kp = pp.tile([P, ROWS], F32, tag="kp", name=f"kp_{ao}")
                    for io in range(IO):
                        nc.tensor.matmul(kp[:], wk[:, io, ao * P:(ao + 1) * P], zT[:, io, :],
                                         start=(io == 0), stop=(io == IO - 1))
                    nc.vector.tensor_scalar(KTs[:, ao, :], kp[:], bks[:, ao:ao + 1], None, OP.add)
                nc.sync.dma_start(out=kt_bd[:], in_=KTs[:])
                nc.gpsimd.collective_compute(
                    "AllGather", OP.bypass, replica_groups=[list(range(NCORES))],
                    ins=[kt_bd[:].opt()], outs=[kt_gd[:].opt()])

                # V next: its AllGather queues on the CC engine right behind K
                nc.scalar.dma_start(out=wv[:], in_=wv_ext[:].rearrange("(io p) d -> p io d", p=P))
                Vs = inp.tile([P, VH, ZB, 512], BF16)
                for nb in range(ZB):
                    vp = pp.tile([P, Ddim], F32, tag="vp", name=f"vp_{nb}", bufs=1)
                    for io in range(IO):
                        for vh in range(VH):
                            nc.tensor.matmul(vp[:, vh * 512:(vh + 1) * 512],
                                             zT[:, io, nb * P:(nb + 1) * P],
                                             wv[:, io, vh * 512:(vh + 1) * 512],
                                             start=(io == 0), stop=(io == IO - 1))
                    for vh in range(VH):
                        nc.vector.tensor_tensor(Vs[:, vh, nb, :], vp[:, vh * 512:(vh + 1) * 512],
                                                bvb[:, vh * 512:(vh + 1) * 512], OP.add)
                nc.sync.dma_start(out=v_bd[:], in_=Vs[:].rearrange("p vh j w -> p (vh j w)"))
                nc.gpsimd.collective_compute(
                    "AllGather", OP.bypass, replica_groups=[list(range(NCORES))],
                    ins=[v_bd[:].opt()], outs=[v_gd[:].opt()])

                # Q^T projection (overlaps the K/V AllGathers)
                nc.scalar.dma_start(out=wq[:], in_=wq_ext[:].rearrange("(io p) d -> p io d", p=P))
                xT = zp.tile([P, IO, ROWS], BF16)
                for io in range(IO):
                    for mb in range(MB):
                        tq = tpp.tile([P, P], BF16, tag="tp", name=f"tq_{mb}_{io}")
                        nc.tensor.transpose(tq[:], xsb[:, mb, io * P:(io + 1) * P], identb[:])
                        nc.vector.tensor_copy(xT[:, io, mb * P:(mb + 1) * P], tq[:])
                for ao in range(AO):
                    qp = pp.tile([P, ROWS], F32, tag="kp", name=f"qp_{ao}")
                    for io in range(IO):
                        nc.tensor.matmul(qp[:], wq[:, io, ao * P:(ao + 1) * P], xT[:, io, :],
                                         start=(io == 0), stop=(io == IO - 1))
                    # fold the softmax 1/sqrt(D) into Q^T
                    nc.vector.tensor_scalar(QT[:, ao, :], qp[:], bqs[:, ao:ao + 1], float(scale),
                                            OP.add, OP.mult)

            # ---------------- Phase 3: attention ----------------
            esT = persist.tile([P, NU, JT, P], BF16)     # P^T chunks for PV
            lacc = persist.tile([P, MB], F32)            # softmax denominators
            acc = persist.tile([P, MB, Ddim], F32)       # normalized output staging
            dmae = (nc.sync, nc.scalar)

            # S pass: S[q,k] = Q^T-chunk (stationary) x K^T tile (moving);
            # exp on scalar emits row-sums via accum_out; PE transposes yield
            # the P^T chunks for PV. Transposes are enqueued one unit behind
            # so the tensor engine never waits on the exp.
            with tc.tile_pool(name="ktp", bufs=4) as ktp, \
                 tc.tile_pool(name="esp", bufs=3) as esp, \
                 tc.tile_pool(name="lpps", bufs=4) as lpps, \
                 tc.tile_pool(name="spp", bufs=2, space="PSUM") as spp, \
                 tc.tile_pool(name="tp2", bufs=2, space="PSUM") as tp2:
                pend = []

                def flush_pend():
                    for (pes, pu) in pend:
                        for kc in range(JT):
                            tp = tp2.tile([P, P], BF16, tag="tp2", name=f"tp2_{pu}_{kc}")
                            nc.tensor.transpose(tp[:], pes[:, kc * P:(kc + 1) * P], identb[:])
                            nc.scalar.activation(esT[:, pu, kc, :], tp[:], AF.Copy)
                    pend.clear()

                for t in range(NT):
                    ktt = ktp.tile([P, AO, SW], BF16, tag="ktt", name=f"ktt_{t}")
                    dmae[t % 2].dma_start(out=ktt[:], in_=kt_gd[t])
                    for m in range(t // 2 + 1):
                        u = UOFF[t] + m
                        sp = spp.tile([P, SW], F32, tag="sp", name=f"sp_{u}")
                        for ao in range(AO):
                            nc.tensor.matmul(sp[:], QT[:, ao, m * P:(m + 1) * P],
                                             ktt[:, ao, :], start=(ao == 0),
                                             stop=(ao == AO - 1))
                        flush_pend()
                        if t == 2 * m:
                            nc.vector.tensor_tensor(sp[:], sp[:], maskA[:], OP.add)
                        elif t == 2 * m + 1:
                            nc.vector.tensor_tensor(sp[:], sp[:], maskB[:], OP.add)
                        es = esp.tile([P, SW], BF16, tag="es", name=f"es_{u}")
                        lp = lpps.tile([P, 1], F32, tag="lp", name=f"lp_{u}")
                        nc.scalar.activation(es[:], sp[:], AF.Exp, accum_out=lp[:])
                        if t == 2 * m:
                            nc.vector.tensor_copy(lacc[:, m:m + 1], lp[:])
                        else:
                            nc.vector.tensor_tensor(lacc[:, m:m + 1], lacc[:, m:m + 1],
                                                    lp[:], OP.add)
                        pend.append((es, u))
                flush_pend()

            # PV pass per value-half, tiles descending so the deepest chunks
            # start immediately after the S pass; psum per query chunk.
            with tc.tile_pool(name="vtp", bufs=3) as vtp, \
                 tc.tile_pool(name="recp", bufs=1) as recp, \
                 tc.tile_pool(name="pvp", bufs=1, space="PSUM") as pvp:
                rec = recp.tile([P, MB], F32)
                nc.vector.reciprocal(rec[:], lacc[:])
                VHW = VW // VH
                for vh in range(VH):
                    pvs = [pvp.tile([P, 512], F32, tag=f"pv{m}", name=f"pv{vh}_{m}")
                           for m in range(MB)]
                    for t in range(NT - 1, -1, -1):
                        vtt = vtp.tile([P, JT, 512], BF16, tag="vtt", name=f"vtt_{vh}_{t}")
                        dmae[t % 2].dma_start(
                            out=vtt[:],
                            in_=v_gd[t][:, vh * VHW:(vh + 1) * VHW].rearrange(
                                "p (j w) -> p j w", w=512))
                        for m in range(t // 2 + 1):
                            u = UOFF[t] + m
                            for kc in range(JT):
                                nc.tensor.matmul(pvs[m][:], esT[:, u, kc, :],
                                                 vtt[:, kc, :],
                                                 start=(t == NT - 1 and kc == 0),
                                                 stop=(t == 2 * m and kc == JT - 1))
                    for m in range(MB):
                        nc.vector.tensor_scalar(acc[:, m, vh * 512:(vh + 1) * 512],
                                                pvs[m][:], rec[:, m:m + 1], None, OP.mult)

            # write out per chunk
            oview = out_ext[:].rearrange("(mb p) v -> p mb v", p=P)
            for m in range(MB):
                nc.sync.dma_start(out=oview[:, m, :], in_=acc[:, m, :])
    nc.compile()
    return nc


_GRAPH_CACHE = {}


def _get_graph(Ldim=L, Ddim=D):
    key = (Ldim, Ddim)
    if key not in _GRAPH_CACHE:
        _GRAPH_CACHE[key] = build_graph(Ldim, Ddim)
    return _GRAPH_CACHE[key]


def kernel(x, z, Wq, bq, Wk, bk, Wv, bv):
    x = np.ascontiguousarray(np.asarray(x, dtype=np.float32)).astype(BF16_NP)
    z = np.ascontiguousarray(np.asarray(z, dtype=np.float32)).astype(BF16_NP)
    Ldim, Ddim = x.shape
    NPART = P
    nc = _get_graph(Ldim, Ddim)
    ROWS = Ldim // NCORES
    common = {
        "Wq": np.ascontiguousarray(np.asarray(Wq, np.float32).astype(BF16_NP)),
        "bq": np.ascontiguousarray(np.asarray(bq, np.float32)),
        "Wk": np.ascontiguousarray(np.asarray(Wk, np.float32).astype(BF16_NP)),
        "bk": np.ascontiguousarray(np.asarray(bk, np.float32)),
        "Wv": np.ascontiguousarray(np.asarray(Wv, np.float32).astype(BF16_NP)),
        "bv": np.ascontiguousarray(np.asarray(bv, np.float32)),
    }
    in_maps = []
    for c in range(NCORES):
        m = dict(common)
        xc = x[c::NCORES]                      # interleaved query rows
        zc = z[ROWS * c:ROWS * (c + 1)]        # contiguous key rows
        m["x"] = np.ascontiguousarray(
            xc.reshape(ROWS // NPART, NPART, Ddim).transpose(1, 0, 2))
        m["z"] = np.ascontiguousarray(
            zc.reshape(ROWS // NPART, NPART, Ddim).transpose(1, 0, 2))
        m["cval"] = np.array([c], dtype=np.float32)
        in_maps.append(m)
    try:
        res = run_bass_kernel_spmd(nc, in_maps, core_ids=list(range(NCORES)))
    except Exception:
        # transient NRT device hiccups have been observed; one retry
        res = run_bass_kernel_spmd(nc, in_maps, core_ids=list(range(NCORES)))
    out = np.empty((Ldim, Ddim), dtype=np.float32)
    for c in range(NCORES):
        out[c::NCORES] = res.results[c]["out"]
    return out


# revision 16
# speedup vs baseline: 1.4048x; 1.0790x over previous
"""Distributed Trainium2 Bass kernel: masked (upper-triangular) attention.

reference (L=4096, D=1024, fp32):
    Q = x @ Wq + bq ; K = z @ Wk + bk ; V = z @ Wv + bv
    S = Q @ K.T ; S[row > col] = -inf
    out = softmax(S / sqrt(D)) @ V

Strategy (8 NeuronCores, one TRN2 chip, SPMD):
  - Query rows dealt round-robin: core c owns rows {r : r % 8 == c}. This
    makes the causal (keep col >= row) footprint IDENTICAL on every core:
    query chunk m (128 local rows = global rows c+8*(128m..)) attends key
    tile t (512 keys) iff 2m <= t -> a uniform static 20-unit schedule that
    skips ~44% of the S/PV work with no per-core addressing.
  - K/V projections sharded over contiguous z blocks (512/core), AllGathered
    in bf16 into Shared-address-space DRAM (K^T as [d,keys], V natural).
  - S computed in [q, k] orientation (Q^T chunk stationary, K^T tile moving
    512-wide); exp on scalar engine emits row-sums via accum_out; P^T for
    the PV matmul obtained with PE transposes of the 128x128 es chunks.
  - Only the two near-diagonal tiles per chunk need masks: two constant
    [128,512] additive (-50) masks built once from an iota + core id.
  - Matmuls in bf16 with fp32 PSUM accumulation.
"""

import math

import ml_dtypes
import numpy as np

BF16_NP = ml_dtypes.bfloat16

import concourse.mybir as mybir
import concourse.tile as tile
from concourse import bacc
from concourse.bass_utils import run_bass_kernel_spmd

F32 = mybir.dt.float32
BF16 = mybir.dt.bfloat16
AF = mybir.ActivationFunctionType
OP = mybir.AluOpType
P = 128
NCORES = 8

L = 4096
D = 1024


def build_graph(Ldim=L, Ddim=D):
    nc = bacc.Bacc("TRN2", target_bir_lowering=False, debug=False, num_devices=NCORES)
    ROWS = Ldim // NCORES        # query rows per core
    MB = ROWS // P               # 128-row query chunks per core (4)
    ZB = ROWS // P               # z-shard 128-row blocks (4)
    SW = ROWS                    # key-tile width == z-shard width (512)
    JT = SW // P                 # 128-key subtiles per key tile (4)
    NT = NCORES                  # one key tile per shard
    IO = Ddim // P               # contraction chunks (8)
    AO = Ddim // P               # d_attn 128-blocks (8)
    VH = Ddim // 512             # 512-wide value column halves (2)
    scale = 1.0 / math.sqrt(Ddim)
    # units (t, m) with 2m <= t; unit index = UOFF[t] + m
    UCNT = [t // 2 + 1 for t in range(NT)]
    UOFF = [sum(UCNT[:t]) for t in range(NT)]
    NU = sum(UCNT)               # 20

    x_ext = nc.declare_dram_parameter("x", [P, MB, Ddim], BF16, isOutput=False)
    z_ext = nc.declare_dram_parameter("z", [P, ZB, Ddim], BF16, isOutput=False)
    wq_ext = nc.declare_dram_parameter("Wq", [Ddim, Ddim], BF16, isOutput=False)
    wk_ext = nc.declare_dram_parameter("Wk", [Ddim, Ddim], BF16, isOutput=False)
    wv_ext = nc.declare_dram_parameter("Wv", [Ddim, Ddim], BF16, isOutput=False)
    bq_ext = nc.declare_dram_parameter("bq", [Ddim], F32, isOutput=False)
    bk_ext = nc.declare_dram_parameter("bk", [Ddim], F32, isOutput=False)
    bv_ext = nc.declare_dram_parameter("bv", [Ddim], F32, isOutput=False)
    cval_ext = nc.declare_dram_parameter("cval", [1], F32, isOutput=False)
    out_ext = nc.declare_dram_parameter("out", [ROWS, Ddim], F32, isOutput=True)

    ident_d = nc.inline_tensor(np.eye(P, dtype=np.float32), name="ident_c")
    identb_d = nc.inline_tensor(np.eye(P, dtype=np.float32), name="identb_c")

    with tile.TileContext(nc) as tc:
        with tc.tile_pool(name="const", bufs=1) as constp, \
             tc.tile_pool(name="persist", bufs=1) as persist, \
             tc.tile_pool(name="dram", bufs=1, space="DRAM") as dram:
            identf = constp.tile([P, P], F32)
            nc.scalar.dma_start(out=identf[:], in_=identb_d.ap())
            identb = constp.tile([P, P], BF16)
            nc.vector.tensor_copy(identb[:], identf[:])
            bvb = constp.tile([P, Ddim], F32)
            nc.scalar.dma_start(out=bvb[:], in_=bv_ext[:].partition_broadcast(P))
            bqs = constp.tile([P, AO], F32)
            nc.scalar.dma_start(out=bqs[:], in_=bq_ext[:].rearrange("(ao p) -> p ao", p=P))
            bks = constp.tile([P, AO], F32)
            nc.scalar.dma_start(out=bks[:], in_=bk_ext[:].rearrange("(ao p) -> p ao", p=P))
            cvb = constp.tile([P, 1], F32)
            nc.scalar.dma_start(out=cvb[:], in_=cval_ext[:].partition_broadcast(P))

            QT = persist.tile([P, AO, ROWS], BF16)
            KH = 2                       # key halves per tile (split K AllGather)
            KW = AO * (ROWS // KH)       # flat K width per partition per half
            VW = ZB * Ddim               # flat V width per partition
            kt_bds = [dram.tile([P, AO, ROWS // KH], BF16, name=f"kt_bd{h}")
                      for h in range(KH)]
            v_bds = [dram.tile([P, VW // VH], BF16, name=f"v_bd{vh}") for vh in range(VH)]
            kt_gds = [dram.tile([NCORES, P, AO, ROWS // KH], BF16, name=f"kt_gd{h}",
                                addr_space="Shared") for h in range(KH)]
            v_gds = [dram.tile([NCORES, P, VW // VH], BF16, name=f"v_gd{vh}",
                               addr_space="Shared") for vh in range(VH)]

            # additive pre-softmax masks for the two near-diagonal tiles of
            # each query chunk: with r = c + 8i + 1024m, keys k = 512t + f:
            #   t == 2m  : keep iff f - 8i - c >= 0        (maskA)
            #   t == 2m+1: keep iff f - 8i - c + 512 >= 0  (maskB)
            maskA = persist.tile([P, SW], F32)
            maskB = persist.tile([P, SW], F32)
            with tc.tile_pool(name="iop", bufs=1) as iop:
                iof = iop.tile([P, SW], F32)
                nc.gpsimd.iota(iof[:], pattern=[[1, SW]], base=0,
                               channel_multiplier=-8,
                               allow_small_or_imprecise_dtypes=True)
                tA = iop.tile([P, SW], F32)
                nc.vector.tensor_scalar(tA[:], iof[:], cvb[:], None, OP.subtract)
                mkA = iop.tile([P, SW], F32)
                nc.vector.tensor_scalar(mkA[:], tA[:], 0.0, None, OP.is_ge)
                nc.vector.tensor_scalar(maskA[:], mkA[:], 1.0, 50.0, OP.subtract, OP.mult)
                tB = iop.tile([P, SW], F32)
                nc.vector.tensor_scalar(tB[:], tA[:], 512.0, None, OP.add)
                mkB = iop.tile([P, SW], F32)
                nc.vector.tensor_scalar(mkB[:], tB[:], 0.0, None, OP.is_ge)
                nc.vector.tensor_scalar(maskB[:], mkB[:], 1.0, 50.0, OP.subtract, OP.mult)

            # ------- Phase 1+2: projections of own shards; K/V AllGathered -------
            # Inputs arrive pre-cast to bf16 from the host: weights DMA straight
            # into their SBUF tiles (no staging/cast), transposes run in bf16.
            with tc.tile_pool(name="inp", bufs=1) as inp, \
                 tc.tile_pool(name="wkv", bufs=1) as wp, \
                 tc.tile_pool(name="zp", bufs=1) as zp, \
                 tc.tile_pool(name="tpp", bufs=2, space="PSUM") as tpp, \
                 tc.tile_pool(name="pp", bufs=2, space="PSUM") as pp:
                zsb = inp.tile([P, ZB, Ddim], BF16)
                nc.sync.dma_start(out=zsb[:], in_=z_ext[:])
                xsb = inp.tile([P, MB, Ddim], BF16)
                nc.sync.dma_start(out=xsb[:], in_=x_ext[:])
                wk = wp.tile([P, IO, Ddim], BF16)
                wv = wp.tile([P, IO, Ddim], BF16)
                wq = wp.tile([P, IO, Ddim], BF16)
                nc.scalar.dma_start(out=wk[:], in_=wk_ext[:].rearrange("(io p) d -> p io d", p=P))
                zT = zp.tile([P, IO, ROWS], BF16)
                for io in range(IO):
                    for nb in range(ZB):
                        tp = tpp.tile([P, P], BF16, tag="tp", name=f"tp_{nb}_{io}")
                        nc.tensor.transpose(tp[:], zsb[:, nb, io * P:(io + 1) * P], identb[:])
                        nc.vector.tensor_copy(zT[:, io, nb * P:(nb + 1) * P], tp[:])

                KTs = inp.tile([P, AO, ROWS], BF16)
                for ao in range(AO):
                    kp = pp.tile([P, ROWS], F32, tag="kp", name=f"kp_{ao}")
                    for io in range(IO):
                        nc.tensor.matmul(kp[:], wk[:, io, ao * P:(ao + 1) * P], zT[:, io, :],
                                         start=(io == 0), stop=(io == IO - 1))
                    nc.vector.tensor_scalar(KTs[:, ao, :], kp[:], bks[:, ao:ao + 1], None, OP.add)
                KHW = ROWS // KH
                for h in range(KH):
                    nc.sync.dma_start(out=kt_bds[h][:], in_=KTs[:, :, h * KHW:(h + 1) * KHW])
                    nc.gpsimd.collective_compute(
                        "AllGather", OP.bypass, replica_groups=[list(range(NCORES))],
                        ins=[kt_bds[h][:].opt()], outs=[kt_gds[h][:].opt()])

                # V next: its AllGathers queue on the CC engine right behind K
                nc.scalar.dma_start(out=wv[:], in_=wv_ext[:].rearrange("(io p) d -> p io d", p=P))
                Vs = inp.tile([P, VH, ZB, 512], BF16)
                for nb in range(ZB):
                    vp = pp.tile([P, Ddim], F32, tag="vp", name=f"vp_{nb}", bufs=1)
                    for io in range(IO):
                        for vh in range(VH):
                            nc.tensor.matmul(vp[:, vh * 512:(vh + 1) * 512],
                                             zT[:, io, nb * P:(nb + 1) * P],
                                             wv[:, io, vh * 512:(vh + 1) * 512],
                                             start=(io == 0), stop=(io == IO - 1))
                    for vh in range(VH):
                        nc.vector.tensor_tensor(Vs[:, vh, nb, :], vp[:, vh * 512:(vh + 1) * 512],
                                                bvb[:, vh * 512:(vh + 1) * 512], OP.add)
                for vh in range(VH):
                    nc.sync.dma_start(out=v_bds[vh][:], in_=Vs[:, vh])
                    nc.gpsimd.collective_compute(
                        "AllGather", OP.bypass, replica_groups=[list(range(NCORES))],
                        ins=[v_bds[vh][:].opt()], outs=[v_gds[vh][:].opt()])

                # Q^T projection (overlaps the K/V AllGathers)
                nc.scalar.dma_start(out=wq[:], in_=wq_ext[:].rearrange("(io p) d -> p io d", p=P))
                xT = zp.tile([P, IO, ROWS], BF16)
                for io in range(IO):
                    for mb in range(MB):
                        tq = tpp.tile([P, P], BF16, tag="tp", name=f"tq_{mb}_{io}")
                        nc.tensor.transpose(tq[:], xsb[:, mb, io * P:(io + 1) * P], identb[:])
                        nc.vector.tensor_copy(xT[:, io, mb * P:(mb + 1) * P], tq[:])
                for ao in range(AO):
                    qp = pp.tile([P, ROWS], F32, tag="kp", name=f"qp_{ao}")
                    for io in range(IO):
                        nc.tensor.matmul(qp[:], wq[:, io, ao * P:(ao + 1) * P], xT[:, io, :],
                                         start=(io == 0), stop=(io == IO - 1))
                    # fold the softmax 1/sqrt(D) into Q^T
                    nc.vector.tensor_scalar(QT[:, ao, :], qp[:], bqs[:, ao:ao + 1], float(scale),
                                            OP.add, OP.mult)

            # ---------------- Phase 3: attention ----------------
            esT = persist.tile([P, NU, JT, P], BF16)     # P^T chunks for PV
            lacc = persist.tile([P, MB], F32)            # softmax denominators
            acc = persist.tile([P, MB, Ddim], F32)       # normalized output staging
            dmae = (nc.sync, nc.scalar)

            # S pass: S[q,k] = Q^T-chunk (stationary) x K^T half-tile (moving);
            # two sub-passes, one per gathered key-half so compute starts
            # right after the first K AllGather lands. Exp on scalar emits
            # row-sums via accum_out; PE transposes yield the P^T chunks for
            # PV, enqueued one unit behind so tensor never waits on the exp.
            KHW = SW // KH
            JH = JT // KH                # 128-key chunks per half (2)
            with tc.tile_pool(name="ktp", bufs=4) as ktp, \
                 tc.tile_pool(name="esp", bufs=4) as esp, \
                 tc.tile_pool(name="lpps", bufs=4) as lpps, \
                 tc.tile_pool(name="spp", bufs=2, space="PSUM") as spp, \
                 tc.tile_pool(name="tp2", bufs=2, space="PSUM") as tp2:
                pend = []

                def flush_pend():
                    for (pes, pu, ph) in pend:
                        for j in range(JH):
                            kc = ph * JH + j
                            tp = tp2.tile([P, P], BF16, tag="tp2", name=f"tp2_{pu}_{kc}")
                            nc.tensor.transpose(tp[:], pes[:, j * P:(j + 1) * P], identb[:])
                            nc.scalar.activation(esT[:, pu, kc, :], tp[:], AF.Copy)
                    pend.clear()

                for h in range(KH):
                    for t in range(NT):
                        ktt = ktp.tile([P, AO, KHW], BF16, tag="ktt", name=f"ktt_{h}_{t}")
                        dmae[t % 2].dma_start(out=ktt[:], in_=kt_gds[h][t])
                        for m in range(t // 2 + 1):
                            u = UOFF[t] + m
                            sp = spp.tile([P, KHW], F32, tag="sp", name=f"sp_{u}_{h}")
                            for ao in range(AO):
                                nc.tensor.matmul(sp[:], QT[:, ao, m * P:(m + 1) * P],
                                                 ktt[:, ao, :], start=(ao == 0),
                                                 stop=(ao == AO - 1))
                            flush_pend()
                            if t == 2 * m:
                                nc.vector.tensor_tensor(sp[:], sp[:],
                                                        maskA[:, h * KHW:(h + 1) * KHW],
                                                        OP.add)
                            elif t == 2 * m + 1:
                                nc.vector.tensor_tensor(sp[:], sp[:],
                                                        maskB[:, h * KHW:(h + 1) * KHW],
                                                        OP.add)
                            es = esp.tile([P, KHW], BF16, tag="es", name=f"es_{u}_{h}")
                            lp = lpps.tile([P, 1], F32, tag="lp", name=f"lp_{u}_{h}")
                            nc.scalar.activation(es[:], sp[:], AF.Exp, accum_out=lp[:])
                            if t == 2 * m and h == 0:
                                nc.vector.tensor_copy(lacc[:, m:m + 1], lp[:])
                            else:
                                nc.vector.tensor_tensor(lacc[:, m:m + 1], lacc[:, m:m + 1],
                                                        lp[:], OP.add)
                            pend.append((es, u, h))
                flush_pend()

            # PV pass per value-half, tiles descending so the deepest chunks
            # start immediately after the S pass; psum per query chunk. Each
            # chunk is normalized (and on the second half, written out) as
            # soon as its accumulation stops, spreading the output DMAs.
            oview = out_ext[:].rearrange("(mb p) v -> p mb v", p=P)
            with tc.tile_pool(name="vtp", bufs=3) as vtp, \
                 tc.tile_pool(name="recp", bufs=1) as recp, \
                 tc.tile_pool(name="pvp", bufs=1, space="PSUM") as pvp:
                rec = recp.tile([P, MB], F32)
                nc.vector.reciprocal(rec[:], lacc[:])
                for vh in range(VH):
                    pvs = [pvp.tile([P, 512], F32, tag=f"pv{m}", name=f"pv{vh}_{m}")
                           for m in range(MB)]
                    for t in range(NT - 1, -1, -1):
                        vtt = vtp.tile([P, JT, 512], BF16, tag="vtt", name=f"vtt_{vh}_{t}")
                        dmae[t % 2].dma_start(out=vtt[:], in_=v_gds[vh][t])
                        for m in range(t // 2 + 1):
                            u = UOFF[t] + m
                            for kc in range(JT):
                                nc.tensor.matmul(pvs[m][:], esT[:, u, kc, :],
                                                 vtt[:, kc, :],
                                                 start=(t == NT - 1 and kc == 0),
                                                 stop=(t == 2 * m and kc == JT - 1))
                        if t % 2 == 0:
                            m = t // 2
                            nc.vector.tensor_scalar(acc[:, m, vh * 512:(vh + 1) * 512],
                                                    pvs[m][:], rec[:, m:m + 1], None, OP.mult)
                            if vh == VH - 1:
                                nc.sync.dma_start(out=oview[:, m, :], in_=acc[:, m, :])
    nc.compile()
    return nc


_GRAPH_CACHE = {}


def _get_graph(Ldim=L, Ddim=D):
    key = (Ldim, Ddim)
    if key not in _GRAPH_CACHE:
        _GRAPH_CACHE[key] = build_graph(Ldim, Ddim)
    return _GRAPH_CACHE[key]


def kernel(x, z, Wq, bq, Wk, bk, Wv, bv):
    x = np.ascontiguousarray(np.asarray(x, dtype=np.float32)).astype(BF16_NP)
    z = np.ascontiguousarray(np.asarray(z, dtype=np.float32)).astype(BF16_NP)
    Ldim, Ddim = x.shape
    NPART = P
    nc = _get_graph(Ldim, Ddim)
    ROWS = Ldim // NCORES
    common = {
        "Wq": np.ascontiguousarray(np.asarray(Wq, np.float32).astype(BF16_NP)),
        "bq": np.ascontiguousarray(np.asarray(bq, np.float32)),
        "Wk": np.ascontiguousarray(np.asarray(Wk, np.float32).astype(BF16_NP)),
        "bk": np.ascontiguousarray(np.asarray(bk, np.float32)),
        "Wv": np.ascontiguousarray(np.asarray(Wv, np.float32).astype(BF16_NP)),
        "bv": np.ascontiguousarray(np.asarray(bv, np.float32)),
    }
    in_maps = []
    for c in range(NCORES):
        m = dict(common)
        xc = x[c::NCORES]                      # interleaved query rows
        zc = z[ROWS * c:ROWS * (c + 1)]        # contiguous key rows
        m["x"] = np.ascontiguousarray(
            xc.reshape(ROWS // NPART, NPART, Ddim).transpose(1, 0, 2))
        m["z"] = np.ascontiguousarray(
            zc.reshape(ROWS // NPART, NPART, Ddim).transpose(1, 0, 2))
        m["cval"] = np.array([c], dtype=np.float32)
        in_maps.append(m)
    try:
        res = run_bass_kernel_spmd(nc, in_maps, core_ids=list(range(NCORES)))
    except Exception:
        # transient NRT device hiccups have been observed; one retry
        res = run_bass_kernel_spmd(nc, in_maps, core_ids=list(range(NCORES)))
    out = np.empty((Ldim, Ddim), dtype=np.float32)
    for c in range(NCORES):
        out[c::NCORES] = res.results[c]["out"]
    return out
